# revision 5
# baseline (speedup 1.0000x reference)
"""Multi-head attention block (QKV proj + causal softmax attention + out-proj
+ residual + LayerNorm) on 8 Trainium2 NeuronCores.

Sharding: phase 1 shards (batch, head-group): core = 4*b + g computes heads
[4g, 4g+4) of batch b. Phase 2 shards (batch, seq): core = 4*b + i computes
output rows [512i, 512(i+1)) of batch b. The reshard between phases happens
on host (concat of per-core outputs).

Device layout notes:
- Activations are kept feature-major ("T layout"): xT = x.T is supplied by the
  host, QT/KT [d_head-major, seq] come straight out of the projection matmuls,
  V is produced in seq-major layout for use as the PV stationary operand.
- Softmax skips max-subtraction (scores are ~N(0,1) for any LN-scale input;
  exp cannot overflow fp32) which allows computing scores transposed
  (S^T [kv, q]) and summing denominators via an appended ones column in V.
"""

import sys

sys.path.insert(0, "/opt/trn_rl_repo")

import numpy as np

import concourse.bass as bass
import concourse.mybir as mybir
import concourse.tile as tile
from concourse.bass_utils import run_bass_kernel_spmd

F32 = mybir.dt.float32
P = 128
B, S, DM = 2, 2048, 1024
NH, DK = 16, 64
HG = 4  # heads per group (phase-1 core)
DG = HG * DK  # 256 features per group
SQ = 512  # q-block width (matmul free dim)
NJ = S // SQ  # 4 q-blocks
NC = S // P  # 16 kv chunks
NKC = DM // P  # 8 contraction chunks for dmodel


def _legalize_waits(nc, max_waits=1):
    """walrus in this container accepts only one sync-wait command per
    instruction; move extra waits onto same-engine drains inserted before."""
    import bass_rust

    counter = 0
    for f in nc.m.functions:
        for b in f.blocks:
            insts = list(b.instructions)
            new_insts = []
            changed = False
            for inst in insts:
                si = inst.sync_info
                if (
                    si is not None
                    and len(si.on_wait) > max_waits
                    and inst.engine != mybir.EngineType.Unassigned
                ):
                    waits = list(si.on_wait)
                    reg_waits = [w for w in waits if w.wait_reg is not None]
                    imm_waits = [w for w in waits if w.wait_reg is None]
                    keep = max(0, max_waits - len(reg_waits))
                    spill = imm_waits[:-keep] if keep else imm_waits
                    tail = imm_waits[-keep:] if keep else []
                    for w in spill:
                        counter += 1
                        d = mybir.InstDrain(name=f"I-waitspill-{id(nc)}-{counter}")
                        d.engine = inst.engine
                        d.sync_info = bass_rust.SyncInfo(on_wait=[w], on_update=[])
                        new_insts.append(d)
                    inst.sync_info = bass_rust.SyncInfo(
                        on_wait=reg_waits + tail, on_update=list(si.on_update)
                    )
                    changed = True
                new_insts.append(inst)
            if changed:
                b.instructions = new_insts


def build_phase1(causal=True):
    """Per core: xT_{q,k,v} [1024, 2048], w_{q,k,v} [1024, 256] ->
    ot [256, 2048] = softmax(QK^T/sqrt(dk)) V, transposed, for 4 heads."""
    nc = bass.Bass(trn_type="TRN2", num_devices=8)
    xtq = nc.dram_tensor("xtq", [DM, S], F32, kind="ExternalInput")
    xtk = nc.dram_tensor("xtk", [DM, S], F32, kind="ExternalInput")
    xtv = nc.dram_tensor("xtv", [DM, S], F32, kind="ExternalInput")
    wq = nc.dram_tensor("wq", [DM, DG], F32, kind="ExternalInput")
    wk = nc.dram_tensor("wk", [DM, DG], F32, kind="ExternalInput")
    wv = nc.dram_tensor("wv", [DM, DG], F32, kind="ExternalInput")
    ot = nc.dram_tensor("ot", [DG, S], F32, kind="ExternalOutput")

    with tile.TileContext(nc) as tc:
        with (
            tc.tile_pool(name="xt", bufs=9) as xt_pool,
            tc.tile_pool(name="w", bufs=2) as w_pool,
            tc.tile_pool(name="kqv", bufs=1) as kqv_pool,
            tc.tile_pool(name="work", bufs=3) as work_pool,
            tc.tile_pool(name="small", bufs=4) as small_pool,
            tc.tile_pool(name="psum", bufs=2, space="PSUM") as psum_pool,
        ):
            ones1 = kqv_pool.tile([1, DK], F32, name="ones1")
            nc.vector.memset(ones1, 1.0)

            # causal mask tiles: mask[t][x, y] = 1.0 if y >= x + 128*t else 0
            masks = []
            for t in range(4):
                mk = kqv_pool.tile([P, SQ], F32, name=f"mask{t}")
                nc.gpsimd.memset(mk, 1.0)
                nc.gpsimd.affine_select(
                    out=mk,
                    in_=mk,
                    compare_op=mybir.AluOpType.is_ge,
                    fill=0.0,
                    base=-128 * t,
                    pattern=[[1, SQ]],
                    channel_multiplier=-1,
                )
                masks.append(mk)

            # ---- projections ----
            kt = {}  # kt[m][n]: [128, 512] rows = heads (2m, 2m+1) interleaved
            qt = {}
            vt = []  # vt[s]: [128, 4, 65] seq-chunk s, per head 64 cols + ones

            for kind, xdram, wdram in (("k", xtk, wk), ("q", xtq, wq), ("v", xtv, wv)):
                w_sb = w_pool.tile([P, NKC, DG], F32, tag="w", name=f"w_{kind}")
                nc.sync.dma_start(
                    out=w_sb, in_=wdram.rearrange("(c p) n -> p c n", p=P)
                )
                xts = []
                for c in range(NKC):
                    xc = xt_pool.tile([P, S], F32, tag="xt", name=f"x_{kind}{c}")
                    nc.sync.dma_start(out=xc, in_=xdram[c * P : (c + 1) * P, :])
                    xts.append(xc)

                if kind in ("k", "q"):
                    dst = kt if kind == "k" else qt
                    for m in range(DG // P):
                        dst[m] = {}
                        for n in range(NJ):
                            ps = psum_pool.tile(
                                [P, SQ], F32, tag="proj", bufs=2, name=f"ps_{kind}{m}{n}"
                            )
                            for c in range(NKC):
                                nc.tensor.matmul(
                                    ps,
                                    lhsT=w_sb[:, c, m * P : (m + 1) * P],
                                    rhs=xts[c][:, n * SQ : (n + 1) * SQ],
                                    start=(c == 0),
                                    stop=(c == NKC - 1),
                                )
                            o = kqv_pool.tile([P, SQ], F32, name=f"{kind}t{m}{n}")
                            if kind == "q":
                                # fold in 1/sqrt(dk)
                                nc.vector.tensor_scalar_mul(o, ps, 1.0 / 8.0)
                            else:
                                nc.vector.tensor_copy(out=o, in_=ps)
                            dst[m][n] = o
                else:
                    for s in range(NC):
                        ps = psum_pool.tile(
                            [P, DG], F32, tag="proj", bufs=2, name=f"ps_v{s}"
                        )
                        for c in range(NKC):
                            nc.tensor.matmul(
                                ps,
                                lhsT=xts[c][:, s * P : (s + 1) * P],
                                rhs=w_sb[:, c, :],
                                start=(c == 0),
                                stop=(c == NKC - 1),
                            )
                        v = kqv_pool.tile([P, HG, DK + 1], F32, name=f"v{s}")
                        nc.gpsimd.memset(v, 1.0)
                        nc.vector.tensor_copy(
                            out=v[:, :, 0:DK],
                            in_=ps.rearrange("p (h d) -> p h d", h=HG),
                        )
                        vt.append(v)

            # ---- attention (4 heads) ----
            for hp in range(2):  # head pair = row block of kt/qt tiles
                for j in range(NJ):
                    nchunks = 4 * (j + 1) if causal else NC
                    pv_ps = []
                    for hl in range(2):
                        pv_ps.append(
                            psum_pool.tile(
                                [DK + 1, SQ],
                                F32,
                                tag=f"pv{hl}",
                                bufs=1,
                                name=f"pv{hp}{j}{hl}",
                            )
                        )
                    for c in range(nchunks):
                        n, o = c // 4, (c % 4) * P
                        for hl in range(2):
                            row = hl * DK
                            s_ps = psum_pool.tile(
                                [P, SQ], F32, tag=f"s{hl}", bufs=2, name=f"s{hp}{j}{c}{hl}"
                            )
                            nc.tensor.matmul(
                                s_ps,
                                lhsT=kt[hp][n][row : row + DK, o : o + P],
                                rhs=qt[hp][j][row : row + DK, :],
                                start=True,
                                stop=True,
                            )
                            e = work_pool.tile(
                                [P, SQ], F32, tag=f"e{hl}", name=f"e{hp}{j}{c}{hl}"
                            )
                            nc.scalar.activation(
                                e, s_ps, mybir.ActivationFunctionType.Exp
                            )
                            if causal and c >= 4 * j:
                                nc.vector.tensor_mul(e, e, masks[c - 4 * j])
                            nc.tensor.matmul(
                                pv_ps[hl],
                                lhsT=vt[c][:, 2 * hp + hl, :],
                                rhs=e,
                                start=(c == 0),
                                stop=(c == nchunks - 1),
                            )
                    for hl in range(2):
                        h = 2 * hp + hl
                        rec = small_pool.tile([1, SQ], F32, tag="rec", name=f"rc{h}{j}")
                        nc.vector.reciprocal(rec, pv_ps[hl][DK : DK + 1, :])
                        # broadcast [1, SQ] across 64 partitions via outer
                        # product with a ones column on the PE
                        rb = psum_pool.tile(
                            [DK, SQ], F32, tag="proj", bufs=2, name=f"rb{h}{j}"
                        )
                        nc.tensor.matmul(rb, lhsT=ones1, rhs=rec, start=True, stop=True)
                        rbs = small_pool.tile([DK, SQ], F32, tag="rbs", name=f"rs{h}{j}")
                        nc.vector.tensor_copy(out=rbs, in_=rb)
                        osb = work_pool.tile(
                            [DK, SQ], F32, tag="osb", name=f"ot{h}{j}"
                        )
                        nc.vector.tensor_mul(osb, pv_ps[hl][0:DK, :], rbs)
                        nc.sync.dma_start(
                            out=ot[h * DK : (h + 1) * DK, j * SQ : (j + 1) * SQ],
                            in_=osb,
                        )

    _legalize_waits(nc)
    return nc


def build_phase2():
    """Per core: ctx [1024, 512] (context^T for 512 q rows, all heads),
    wfc [1024, 1024], xq [512, 1024] -> out [512, 1024] = LN(ctx^T@wfc + xq)."""
    nc = bass.Bass(trn_type="TRN2", num_devices=8)
    ctx = nc.dram_tensor("ctx", [DM, SQ], F32, kind="ExternalInput")
    wfc = nc.dram_tensor("wfc", [DM, DM], F32, kind="ExternalInput")
    xq = nc.dram_tensor("xq", [SQ, DM], F32, kind="ExternalInput")
    gamma = nc.dram_tensor("gamma", [DM], F32, kind="ExternalInput")
    beta = nc.dram_tensor("beta", [DM], F32, kind="ExternalInput")
    out = nc.dram_tensor("out", [SQ, DM], F32, kind="ExternalOutput")

    with tile.TileContext(nc) as tc:
        with (
            tc.tile_pool(name="big", bufs=1) as big_pool,
            tc.tile_pool(name="work", bufs=3) as work_pool,
            tc.tile_pool(name="small", bufs=4) as small_pool,
            tc.tile_pool(name="psum", bufs=2, space="PSUM") as psum_pool,
        ):
            ctx_sb = big_pool.tile([P, NKC, SQ], F32, name="ctx_sb")
            nc.sync.dma_start(out=ctx_sb, in_=ctx.rearrange("(c p) q -> p c q", p=P))
            wfc_sb = big_pool.tile([P, NKC, DM], F32, name="wfc_sb")
            nc.sync.dma_start(out=wfc_sb, in_=wfc.rearrange("(c p) n -> p c n", p=P))

            gb = big_pool.tile([P, DM], F32, name="gb")
            nc.gpsimd.dma_start(
                out=gb,
                in_=bass.AP(tensor=gamma, offset=0, ap=[[0, P], [1, DM]]),
            )
            bb = big_pool.tile([P, DM], F32, name="bb")
            nc.gpsimd.dma_start(
                out=bb,
                in_=bass.AP(tensor=beta, offset=0, ap=[[0, P], [1, DM]]),
            )
            eps = big_pool.tile([P, 1], F32, name="eps")
            nc.vector.memset(eps, 1e-5)

            for qc in range(SQ // P):
                xq_sb = work_pool.tile([P, DM], F32, tag="xq", name=f"xq{qc}")
                nc.sync.dma_start(out=xq_sb, in_=xq[qc * P : (qc + 1) * P, :])
                y = work_pool.tile([P, DM], F32, tag="y", name=f"y{qc}")
                for half in range(2):
                    ps = psum_pool.tile(
                        [P, SQ], F32, tag="fc", bufs=3, name=f"fc{qc}{half}"
                    )
                    for c in range(NKC):
                        nc.tensor.matmul(
                            ps,
                            lhsT=ctx_sb[:, c, qc * P : (qc + 1) * P],
                            rhs=wfc_sb[:, c, half * SQ : (half + 1) * SQ],
                            start=(c == 0),
                            stop=(c == NKC - 1),
                        )
                    nc.vector.tensor_add(
                        out=y[:, half * SQ : (half + 1) * SQ],
                        in0=ps,
                        in1=xq_sb[:, half * SQ : (half + 1) * SQ],
                    )
                # layer norm over the free dim (1024 = 2 bn subgroups of 512)
                stats = small_pool.tile(
                    [P, 2, nc.vector.BN_STATS_DIM], F32, tag="st", name=f"st{qc}"
                )
                yg = y.rearrange("p (g d) -> p g d", g=2)
                for g in range(2):
                    nc.vector.bn_stats(out=stats[:, g, :], in_=yg[:, g, :])
                mv = small_pool.tile(
                    [P, nc.vector.BN_AGGR_DIM], F32, tag="mv", name=f"mv{qc}"
                )
                nc.vector.bn_aggr(out=mv, in_=stats)
                rstd = small_pool.tile([P, 1], F32, tag="rstd", name=f"rstd{qc}")
                nc.scalar.activation(
                    out=rstd,
                    in_=mv[:, 1:2],
                    func=mybir.ActivationFunctionType.Sqrt,
                    bias=eps,
                )
                nc.vector.reciprocal(out=rstd, in_=rstd)
                t = work_pool.tile([P, DM], F32, tag="t", name=f"t{qc}")
                nc.vector.tensor_scalar(
                    t,
                    y,
                    mv[:, 0:1],
                    rstd,
                    mybir.AluOpType.subtract,
                    mybir.AluOpType.mult,
                )
                o = work_pool.tile([P, DM], F32, tag="o", name=f"o{qc}")
                nc.vector.tensor_mul(o, t, gb)
                nc.vector.tensor_add(out=o, in0=o, in1=bb)
                nc.sync.dma_start(out=out[qc * P : (qc + 1) * P, :], in_=o)

    _legalize_waits(nc)
    return nc


_cache = {}


def _get_nc(which, causal=True):
    key = (which, causal)
    if key not in _cache:
        _cache[key] = build_phase1(causal) if which == 1 else build_phase2()
    return _cache[key]


def kernel(
    input_q,
    input_k,
    input_v,
    attn_mask,
    W_Q,
    W_K,
    W_V,
    W_fc,
    ln_gamma,
    ln_beta,
    _trace=False,
):
    f32 = np.float32
    input_q = np.asarray(input_q, f32)
    input_k = np.asarray(input_k, f32)
    input_v = np.asarray(input_v, f32)
    W_Q = np.asarray(W_Q, f32)
    W_K = np.asarray(W_K, f32)
    W_V = np.asarray(W_V, f32)
    W_fc = np.asarray(W_fc, f32)
    ln_gamma = np.asarray(ln_gamma, f32)
    ln_beta = np.asarray(ln_beta, f32)

    mask = np.asarray(attn_mask)
    causal_ref = np.triu(np.ones((S, S), bool), k=1)
    if all(np.array_equal(mask[b], causal_ref) for b in range(mask.shape[0])):
        causal = True
    elif not mask.any():
        causal = False
    else:
        raise NotImplementedError("only causal or empty attention masks supported")

    xt = {}
    for b in range(B):
        xt[("q", b)] = np.ascontiguousarray(input_q[b].T)
        xt[("k", b)] = np.ascontiguousarray(input_k[b].T)
        xt[("v", b)] = np.ascontiguousarray(input_v[b].T)

    in_maps1 = []
    for core in range(8):
        b, g = divmod(core, 4)
        sl = slice(g * DG, (g + 1) * DG)
        in_maps1.append(
            {
                "xtq": xt[("q", b)],
                "xtk": xt[("k", b)],
                "xtv": xt[("v", b)],
                "wq": np.ascontiguousarray(W_Q[:, sl]),
                "wk": np.ascontiguousarray(W_K[:, sl]),
                "wv": np.ascontiguousarray(W_V[:, sl]),
            }
        )
    nc1 = _get_nc(1, causal)
    res1 = run_bass_kernel_spmd(
        nc1, in_maps1, core_ids=list(range(8)), trace=_trace
    )
    ots = [res1.results[c]["ot"] for c in range(8)]

    in_maps2 = []
    for core in range(8):
        b, i = divmod(core, 4)
        qsl = slice(i * SQ, (i + 1) * SQ)
        ctx = np.ascontiguousarray(
            np.concatenate([ots[4 * b + g][:, qsl] for g in range(4)], axis=0)
        )
        in_maps2.append(
            {
                "ctx": ctx,
                "wfc": W_fc,
                "xq": np.ascontiguousarray(input_q[b, qsl, :]),
                "gamma": ln_gamma,
                "beta": ln_beta,
            }
        )
    nc2 = _get_nc(2)
    res2 = run_bass_kernel_spmd(
        nc2, in_maps2, core_ids=list(range(8)), trace=_trace
    )

    out = np.empty((B, S, DM), f32)
    for core in range(8):
        b, i = divmod(core, 4)
        out[b, i * SQ : (i + 1) * SQ, :] = res2.results[core]["out"]

    kernel.last_exec_ns = (res1.exec_time_ns, res2.exec_time_ns)
    return out
